# revision 14
# baseline (speedup 1.0000x reference)
"""GAT (2-layer graph attention network + mean-pool + FC) on 8 Trainium2 cores.

Strategy
--------
Host: permute nodes by in-degree into tiles of 127 nodes (+1 pad slot at
partition 127, which keeps the tail of every bulk gather on a non-negative
index), shard tiles across 8 cores (dst partitioning), and precompute per-edge
gather index streams.

Device (SPMD, one NEFF on 8 cores):
  - node-feature tables in HBM, bf16 rows of 128 elems (256B):
    [h(64) | alpha_src(4) | alpha_dst(4) | pad]; the alpha projections are
    folded into the weights (W @ a), so one matmul yields h and both alphas.
  - layer-1 projection is replicated (each core projects all nodes from a
    per-core-permuted featT so its own tiles come first -> uniform program).
  - edge phase: per dst-tile [128 nodes x D slots], one bulk dma_gather pulls
    each in-edge's source row; attention is computed densely per (node, slot):
    s = a_src[src] + a_dst[dst] (free-axis broadcast), w = exp(leaky_relu(s)),
    x = sum_d w*h / sum_d w.  Padding slots point at a dummy table row with
    a_src = -60 => w ~ e^-60 ~ 0.
  - layer boundary: per-tile PE transpose + projection of x1, slices
    AllGathered into the layer-2 table (layer-2 gather indices use the
    AllGather layout).
  - pooling: per-graph one-hot matmul accumulated in PSUM; each core emits a
    partial (pooled/cnt) @ fc_W; host sums partials and adds fc_b.
"""

import numpy as np
import ml_dtypes

bf16 = ml_dtypes.bfloat16

# problem constants
N_NODES = 50000
N_EDGES = 800000
N_FEAT = 128
N_HID = 64
HEADS = 4
HEAD_DIM = 16
N_CLASS = 10
NUM_GRAPHS = 64
NEG_SLOPE = 0.2

N_CORES = 8
TPC = 50                      # tiles per core
NT = N_CORES * TPC            # 400 global tiles
RPT = 127                     # real nodes per tile (partition 127 is pad)
NPAD = NT * 128               # 51200 node slots
NROW = NPAD + 1               # tables have one extra dummy row
DUMMY = NPAD                  # dummy table row / sentinel slot
BASE = 32768                  # gather base offset (int16 idx wraps around it)
SPC = TPC * 128               # 6400 slots per core
AG_CHUNKS = [(0, 13), (13, 26), (26, 38), (38, 46), (46, 50)]


# ----------------------------------------------------------------- host prep

def _prep(feat, W1, a1_src, a1_dst, b1, W2, a2_src, a2_dst, b2, fc_W, fc_b,
          edge_index, batch):
    src = np.asarray(edge_index[0], dtype=np.int64)
    dst = np.asarray(edge_index[1], dtype=np.int64)
    batch = np.asarray(batch, dtype=np.int64)
    feat = np.asarray(feat, dtype=np.float32)

    deg = np.bincount(dst, minlength=N_NODES)
    order = np.argsort(-deg, kind="stable")          # node ids, desc degree

    # slot position k = 1024*i + 128*c + p ; real nodes at p in [0, 127)
    # real-rank r (0..N_NODES-1) -> position: i = r // 1016, within = r % 1016,
    # c = within // 127, p = within % 127
    rr = np.arange(N_NODES)
    pos_of_rank = ((rr // 1016) * 1024 + ((rr % 1016) // RPT) * 128
                   + (rr % 1016) % RPT)
    slot_node = np.full(NPAD, -1, dtype=np.int64)    # position -> node id
    slot_node[pos_of_rank] = order
    node_pos = np.empty(N_NODES, dtype=np.int64)     # node id -> position
    node_pos[order] = pos_of_rank

    deg_slot = np.zeros(NPAD, dtype=np.int64)
    deg_slot[pos_of_rank] = deg[order]

    Ds = []
    for i in range(TPC):
        Ds.append(int(max(1, deg_slot[1024 * i:1024 * (i + 1)].max())))

    ks = np.arange(NPAD)
    k_i = ks // 1024
    k_c = (ks % 1024) // 128
    k_p = ks % 128

    # r2: chunked-AllGather table row of position k
    # chunk q covers tile range [i0, i1); within chunk: [c][i-i0][p]
    r2map = np.empty(NROW, dtype=np.int64)
    off = 0
    for (i0, i1) in AG_CHUNKS:
        L = i1 - i0
        m = (k_i >= i0) & (k_i < i1)
        r2map[ks[m]] = (off + k_c[m] * (L * 128)
                        + (k_i[m] - i0) * 128 + k_p[m])
        off += N_CORES * L * 128
    r2map[DUMMY] = DUMMY

    # per-core projection order: own tiles (c, 0..TPC-1) first, then others
    g_of = np.empty((N_CORES, N_CORES, TPC), dtype=np.int64)
    for c in range(N_CORES):
        g_of[c, c, :] = np.arange(TPC)
        g = TPC
        for c2 in range(N_CORES):
            if c2 == c:
                continue
            g_of[c, c2, :] = np.arange(g, g + TPC)
            g += TPC

    # r1 per core: local table row of position k:  r1 = p*NT + g
    r1map = np.empty((N_CORES, NROW), dtype=np.int64)
    for c in range(N_CORES):
        r1map[c, :NPAD] = k_p * NT + g_of[c, k_c, k_i]
        r1map[c, DUMMY] = DUMMY

    # per-dst edge lists -> SRC position matrix [NPAD, Dmax]
    Dmax = max(Ds)
    kd = node_pos[dst]
    ksrc = node_pos[src]
    ordr = np.argsort(kd, kind="stable")
    kd_s = kd[ordr]
    ks_s = ksrc[ordr]
    starts = np.searchsorted(kd_s, np.arange(NPAD))
    dpos = np.arange(N_EDGES) - starts[kd_s]
    SRC = np.full((NPAD, Dmax), DUMMY, dtype=np.int64)
    SRC[kd_s, dpos] = ks_s

    # idx streams (int16, relative to BASE) in gather order g' = d*128 + p:
    # idx tile [16, 8D] wrapped (val(t*16+q) at [q, t]), replicated to 128.
    SD = sum(Ds)
    idx1 = np.empty((N_CORES, 128, 8 * SD), dtype=np.int16)
    idx2 = np.empty((N_CORES, 128, 8 * SD), dtype=np.int16)
    bat = np.empty((N_CORES, 128, TPC), dtype=bf16)
    off = 0
    for i in range(TPC):
        D = Ds[i]
        for c in range(N_CORES):
            base_k = 1024 * i + 128 * c
            blk = SRC[base_k:base_k + 128, :D]           # [128(p), D]
            r2v = (r2map[blk] - BASE).astype(np.int16)
            r1v = (r1map[c][blk] - BASE).astype(np.int16)
            t2 = np.tile(r2v.T.reshape(8 * D, 16).T, (8, 1))
            t1 = np.tile(r1v.T.reshape(8 * D, 16).T, (8, 1))
            idx2[c][:, 8 * off:8 * (off + D)] = t2
            idx1[c][:, 8 * off:8 * (off + D)] = t1
            nodes = slot_node[base_k:base_k + 128]
            bv = np.where(nodes >= 0,
                          batch[np.clip(nodes, 0, N_NODES - 1)],
                          NUM_GRAPHS).astype(np.float32)
            bat[c][:, i] = bv.astype(bf16)
        off += D

    # featT per core: columns ordered by per-core projection tile order
    feat_slot = np.zeros((NPAD, N_FEAT), dtype=np.float32)
    valid = slot_node >= 0
    feat_slot[valid] = feat[slot_node[valid]]
    feat_slot_bf = np.ascontiguousarray(feat_slot.astype(bf16))
    featT = np.empty((N_CORES, N_FEAT, NPAD), dtype=bf16)
    base128 = np.arange(128)
    for c in range(N_CORES):
        kidx = np.empty(NPAD, dtype=np.int64)
        g = 0
        for i in range(TPC):
            kidx[128 * g:128 * (g + 1)] = 1024 * i + 128 * c + base128
            g += 1
        for c2 in range(N_CORES):
            if c2 == c:
                continue
            for i in range(TPC):
                kidx[128 * g:128 * (g + 1)] = 1024 * i + 128 * c2 + base128
                g += 1
        featT[c] = np.ascontiguousarray(feat_slot_bf[kidx].T)

    # folded weights
    def a_mat(a):
        m = np.zeros((N_HID, HEADS), dtype=np.float64)
        a = np.asarray(a, dtype=np.float64)
        for h in range(HEADS):
            m[HEAD_DIM * h:HEAD_DIM * (h + 1), h] = a[h]
        return m

    W1_ = np.asarray(W1, dtype=np.float64)
    W2_ = np.asarray(W2, dtype=np.float64)
    W1e = np.concatenate(
        [W1_, W1_ @ a_mat(a1_src), W1_ @ a_mat(a1_dst)], axis=1).astype(bf16)
    W2e = np.concatenate(
        [W2_, W2_ @ a_mat(a2_src), W2_ @ a_mat(a2_dst)], axis=1).astype(bf16)

    B1T = np.ascontiguousarray(
        np.broadcast_to(np.asarray(b1, np.float32), (128, N_HID)))
    B2T = np.ascontiguousarray(
        np.broadcast_to(np.asarray(b2, np.float32), (128, N_HID)))
    GIOTA = np.ascontiguousarray(np.broadcast_to(
        np.arange(NUM_GRAPHS, dtype=np.float32),
        (128, NUM_GRAPHS)).astype(bf16))
    IDENT = np.eye(128, dtype=np.float32).astype(bf16)
    cnt = np.bincount(batch, minlength=NUM_GRAPHS).astype(np.float32)
    CNTINV = np.ascontiguousarray(np.broadcast_to(
        (1.0 / np.maximum(cnt, 1.0)).astype(np.float32),
        (N_HID, NUM_GRAPHS)))
    FCW = np.asarray(fc_W, dtype=np.float32)
    fcb = np.asarray(fc_b, dtype=np.float32)
    C02 = np.full((128, 1), NEG_SLOPE, dtype=np.float32).astype(bf16)

    in_maps = []
    for c in range(N_CORES):
        in_maps.append({
            "featT": featT[c], "idx1": idx1[c], "idx2": idx2[c],
            "bat": np.ascontiguousarray(bat[c]),
            "W1e": W1e, "W2e": W2e, "B1T": B1T, "B2T": B2T,
            "GIOTA": GIOTA, "IDENT": IDENT, "CNTINV": CNTINV, "FCW": FCW,
            "C02": C02,
        })
    return tuple(Ds), in_maps, fcb


# ------------------------------------------------------------ device program

_BUILD_CACHE = {}


def _build(Ds):
    import concourse.bacc as bacc
    import concourse.mybir as mybir
    import concourse.tile as tile

    if Ds in _BUILD_CACHE:
        return _BUILD_CACHE[Ds]

    SD = sum(Ds)
    Dmax = max(Ds)
    f32 = mybir.dt.float32
    b16 = mybir.dt.bfloat16

    nc = bacc.Bacc("TRN2", target_bir_lowering=False, debug=False,
                   num_devices=N_CORES, num_swdge_queues=4)
    qn_state = [0]
    featT = nc.dram_tensor("featT", [N_FEAT, NPAD], b16, kind="ExternalInput")
    idx1 = nc.dram_tensor("idx1", [128, 8 * SD], mybir.dt.int16,
                          kind="ExternalInput")
    idx2 = nc.dram_tensor("idx2", [128, 8 * SD], mybir.dt.int16,
                          kind="ExternalInput")
    bat = nc.dram_tensor("bat", [128, TPC], b16, kind="ExternalInput")
    W1e_t = nc.dram_tensor("W1e", [N_FEAT, 72], b16, kind="ExternalInput")
    W2e_t = nc.dram_tensor("W2e", [N_HID, 72], b16, kind="ExternalInput")
    B1T_t = nc.dram_tensor("B1T", [128, N_HID], f32, kind="ExternalInput")
    B2T_t = nc.dram_tensor("B2T", [128, N_HID], f32, kind="ExternalInput")
    GIOTA_t = nc.dram_tensor("GIOTA", [128, NUM_GRAPHS], b16,
                             kind="ExternalInput")
    IDENT_t = nc.dram_tensor("IDENT", [128, 128], b16, kind="ExternalInput")
    CNTINV_t = nc.dram_tensor("CNTINV", [N_HID, NUM_GRAPHS], f32,
                              kind="ExternalInput")
    FCW_t = nc.dram_tensor("FCW", [N_HID, N_CLASS], f32, kind="ExternalInput")
    C02_t = nc.dram_tensor("C02", [128, 1], b16, kind="ExternalInput")
    out_part = nc.dram_tensor("out_part", [NUM_GRAPHS, N_CLASS], f32,
                              kind="ExternalOutput")

    table1 = nc.dram_tensor("table1", [NROW, 128], b16, kind="Internal")
    table2 = nc.dram_tensor("table2", [NROW, 128], b16, kind="Internal",
                            addr_space="Shared")
    cc_in = nc.dram_tensor("cc_in", [SPC, 128], b16, kind="Internal")

    with tile.TileContext(nc) as tc:
        with tc.tile_pool(name="consts", bufs=1) as cpool, \
             tc.tile_pool(name="persist", bufs=1) as ppool:
            W1e_sb = cpool.tile([N_FEAT, 72], b16)
            nc.sync.dma_start(W1e_sb[:], W1e_t[:])
            W2e_sb = cpool.tile([N_HID, 72], b16)
            nc.sync.dma_start(W2e_sb[:], W2e_t[:])
            B1T_sb = cpool.tile([128, N_HID], f32)
            nc.sync.dma_start(B1T_sb[:], B1T_t[:])
            B2T_sb = cpool.tile([128, N_HID], f32)
            nc.sync.dma_start(B2T_sb[:], B2T_t[:])
            GIOTA_sb = cpool.tile([128, NUM_GRAPHS], b16)
            nc.sync.dma_start(GIOTA_sb[:], GIOTA_t[:])
            IDENT_sb = cpool.tile([128, 128], b16)
            nc.sync.dma_start(IDENT_sb[:], IDENT_t[:])
            CNTINV_sb = cpool.tile([N_HID, NUM_GRAPHS], f32)
            nc.sync.dma_start(CNTINV_sb[:], CNTINV_t[:])
            FCW_sb = cpool.tile([N_HID, N_CLASS], f32)
            nc.sync.dma_start(FCW_sb[:], FCW_t[:])
            bat_sb = cpool.tile([128, TPC], b16)
            nc.sync.dma_start(bat_sb[:], bat[:])
            C02_sb = cpool.tile([128, 1], b16)
            nc.sync.dma_start(C02_sb[:], C02_t[:])

            # dummy-row patch: h = 0, a_src = -60, a_dst = 0
            dpat = cpool.tile([1, 128], b16)
            nc.vector.memset(dpat[:], 0.0)
            nc.vector.memset(dpat[:, 64:68], -60.0)
            nc.sync.dma_start(table1[DUMMY:DUMMY + 1, :], dpat[:])
            nc.sync.dma_start(table2[DUMMY:DUMMY + 1, :], dpat[:])

            x1T_all = ppool.tile([N_HID, SPC], b16)
            adst1_all = ppool.tile([128, TPC * 4], b16)
            adst2_all = ppool.tile([128, TPC * 4], b16)

            # -------- phase A: layer-1 projection (replicated, NT tiles)
            t1v = table1[0:NPAD, :].rearrange("(p g) e -> p g e", p=128)
            with tc.tile_pool(name="projA", bufs=4) as apool, \
                 tc.tile_pool(name="psumA", bufs=8, space="PSUM") as apsum:
                BT = 16
                for b0 in range(0, NT, BT):
                    blen = min(BT, NT - b0)
                    chunk = apool.tile([N_FEAT, BT * 128], b16, tag="chunk")
                    nc.sync.dma_start(
                        chunk[:, :blen * 128],
                        featT[:, 128 * b0:128 * (b0 + blen)])
                    stag = apool.tile([128, BT * 72], b16, tag="stag")
                    for q0 in range(0, blen, 4):
                        qlen = min(4, blen - q0)
                        ps = apsum.tile([128, 4 * 72], f32, space="PSUM",
                                        tag="psA")
                        for t in range(qlen):
                            g = b0 + q0 + t
                            nc.tensor.matmul(
                                ps[:, 72 * t:72 * (t + 1)],
                                lhsT=chunk[:, 128 * (q0 + t):
                                           128 * (q0 + t + 1)],
                                rhs=W1e_sb[:], start=True, stop=True)
                            if g < TPC:
                                nc.scalar.copy(
                                    adst1_all[:, 4 * g:4 * (g + 1)],
                                    ps[:, 72 * t + 68:72 * t + 72])
                        if (q0 // 4) % 2 == 0:
                            nc.vector.tensor_copy(
                                stag[:, 72 * q0:72 * (q0 + qlen)],
                                ps[:, :72 * qlen])
                        else:
                            nc.scalar.copy(
                                stag[:, 72 * q0:72 * (q0 + qlen)],
                                ps[:, :72 * qlen])
                    nc.sync.dma_start(
                        t1v[:, b0:b0 + blen, 0:72],
                        stag[:, :blen * 72].rearrange(
                            "p (g e) -> p g e", g=blen))

            # per-tile edge aggregation (dense per-node slots)
            def edge_tile(epool, table, idx_t, off8, D, adst_all, i, BT_sb):
                idx_sb = epool.tile([128, 8 * Dmax], mybir.dt.int16,
                                    tag="idx")
                nc.sync.dma_start(idx_sb[:, :8 * D],
                                  idx_t[:, off8:off8 + 8 * D])
                g_t = epool.tile([128, Dmax * 128], b16, tag="gt")
                gv = g_t[:, :D * 128].rearrange("p (d e) -> p d e", d=D)
                # chunked multi-queue gather (single_packet caps at 1024 idx)
                for c0 in range(0, D, 8):
                    cl = min(8, D - c0)
                    nc.gpsimd.dma_gather(
                        out_ap=gv[:, c0:c0 + cl, :],
                        in_ap=table[BASE:, :],
                        idxs_ap=idx_sb[:, 8 * c0:8 * (c0 + cl)],
                        num_idxs=128 * cl, num_idxs_reg=128 * cl,
                        elem_size=128, single_packet=True,
                        queue_num=qn_state[0])
                    qn_state[0] = (qn_state[0] + 1) % 4
                s = epool.tile([128, Dmax * 4], b16, tag="s")
                nc.vector.tensor_tensor(
                    out=s[:, :D * 4].rearrange("p (d h) -> p d h", d=D),
                    in0=gv[:, :, 64:68],
                    in1=adst_all[:, 4 * i:4 * (i + 1)].unsqueeze(1)
                        .broadcast_to([128, D, 4]),
                    op=mybir.AluOpType.add)
                t_ = epool.tile([128, Dmax * 4], b16, tag="t_")
                nc.vector.tensor_tensor(
                    out=t_[:, :D * 4], in0=s[:, :D * 4],
                    in1=C02_sb[:, 0:1].to_broadcast([128, D * 4]),
                    op=mybir.AluOpType.mult)
                s2 = epool.tile([128, Dmax * 4], b16, tag="s2")
                nc.vector.tensor_tensor(out=s2[:, :D * 4], in0=s[:, :D * 4],
                                        in1=t_[:, :D * 4],
                                        op=mybir.AluOpType.max)
                # combined [h*w | w] tile, folded log2 along d (unit stride)
                mw = epool.tile([128, Dmax * 68], b16, tag="mw")
                mv = mw[:, :D * 68].rearrange("p (d e) -> p d e", d=D)
                nc.scalar.activation(mv[:, :, 64:68], s2[:, :D * 4],
                                     mybir.ActivationFunctionType.Exp)
                nc.vector.tensor_tensor(
                    out=mv[:, :, 0:64].rearrange("p d (h c) -> p d h c",
                                                 h=HEADS),
                    in0=gv[:, :, 0:64].rearrange(
                        "p d (h c) -> p d h c", h=HEADS),
                    in1=mv[:, :, 64:68].unsqueeze(3)
                        .broadcast_to([128, D, 4, 16]),
                    op=mybir.AluOpType.mult)
                cur = D
                while cur > 1:
                    k = (cur + 1) // 2
                    r = cur - k
                    nc.vector.tensor_tensor(
                        out=mw[:, :r * 68], in0=mw[:, :r * 68],
                        in1=mw[:, k * 68:cur * 68],
                        op=mybir.AluOpType.add)
                    cur = k
                rcp = epool.tile([128, 4], f32, tag="rcp")
                nc.vector.reciprocal(rcp[:], mw[:, 64:68])
                xm = epool.tile([128, 64], f32, tag="xm")
                nc.vector.tensor_tensor(
                    out=xm[:].rearrange("p (h c) -> p h c", h=HEADS),
                    in0=mw[:, 0:64].rearrange("p (h c) -> p h c", h=HEADS),
                    in1=rcp[:].unsqueeze(2).broadcast_to([128, 4, 16]),
                    op=mybir.AluOpType.mult)
                xb = epool.tile([128, 64], f32, tag="xb")
                nc.vector.tensor_tensor(out=xb[:], in0=xm[:], in1=BT_sb[:],
                                        op=mybir.AluOpType.add)
                x = epool.tile([128, 64], b16, tag="x")
                nc.scalar.activation(x[:], xb[:],
                                     mybir.ActivationFunctionType.Relu)
                return x

            # -------- phases B+C: layer-1 edge + layer-2 projection
            # per tile, with the layer-2-table AllGather chunked in
            with tc.tile_pool(name="edge1", bufs=6) as epool, \
                 tc.tile_pool(name="psumE1", bufs=2, space="PSUM") as epsum:
                off8 = 0
                ag_off = 0
                for i in range(TPC):
                    D = Ds[i]
                    x = edge_tile(epool, table1, idx1, off8, D,
                                  adst1_all, i, B1T_sb)
                    psT = epsum.tile([N_HID, 128], b16, space="PSUM",
                                     tag="psT")
                    nc.tensor.transpose(psT[:], x[:], IDENT_sb[:])
                    nc.scalar.copy(
                        x1T_all[:, 128 * i:128 * (i + 1)], psT[:])
                    psH = epsum.tile([128, 72], f32, space="PSUM", tag="psH")
                    nc.tensor.matmul(
                        psH[:], lhsT=x1T_all[:, 128 * i:128 * (i + 1)],
                        rhs=W2e_sb[:], start=True, stop=True)
                    h2 = epool.tile([128, 128], b16, tag="h2")
                    nc.scalar.copy(h2[:, 0:72], psH[:])
                    nc.scalar.copy(adst2_all[:, 4 * i:4 * (i + 1)],
                                   psH[:, 68:72])
                    nc.sync.dma_start(cc_in[128 * i:128 * (i + 1), :], h2[:])
                    off8 += 8 * D
                    for (i0, i1) in AG_CHUNKS:
                        if i == i1 - 1:
                            L = i1 - i0
                            nc.gpsimd.collective_compute(
                                "AllGather", mybir.AluOpType.bypass,
                                replica_groups=[list(range(N_CORES))],
                                ins=[cc_in[128 * i0:128 * i1, :]],
                                outs=[table2[ag_off:
                                             ag_off + N_CORES * L * 128, :]])
                            ag_off += N_CORES * L * 128

            # -------- phase E: layer-2 edge + pooling
            with tc.tile_pool(name="edge2", bufs=6) as epool, \
                 tc.tile_pool(name="psumP", bufs=1, space="PSUM") as ppsum:
                ps_pool = ppsum.tile([N_HID, NUM_GRAPHS], f32, space="PSUM")
                off8 = 0
                for i in range(TPC):
                    D = Ds[i]
                    x = edge_tile(epool, table2, idx2, off8, D,
                                  adst2_all, i, B2T_sb)
                    P_pool = epool.tile([128, NUM_GRAPHS], b16, tag="pp")
                    nc.vector.tensor_tensor(
                        out=P_pool[:],
                        in0=bat_sb[:, i:i + 1].to_broadcast(
                            [128, NUM_GRAPHS]),
                        in1=GIOTA_sb[:], op=mybir.AluOpType.is_equal)
                    nc.tensor.matmul(ps_pool[:], lhsT=x[:], rhs=P_pool[:],
                                     start=(i == 0), stop=(i == TPC - 1))
                    off8 += 8 * D

                # -------- phase F: scale + FC
                pooledS = epool.tile([N_HID, NUM_GRAPHS], f32)
                nc.vector.tensor_tensor(out=pooledS[:], in0=ps_pool[:],
                                        in1=CNTINV_sb[:],
                                        op=mybir.AluOpType.mult)
                ps_o = ppsum.tile([NUM_GRAPHS, N_CLASS], f32, space="PSUM")
                nc.tensor.matmul(ps_o[:], lhsT=pooledS[:], rhs=FCW_sb[:],
                                 start=True, stop=True)
                outsb = epool.tile([NUM_GRAPHS, N_CLASS], f32)
                nc.vector.tensor_copy(outsb[:], ps_o[:])
                nc.sync.dma_start(out_part[:], outsb[:])

    nc.compile()
    _BUILD_CACHE[Ds] = nc
    return nc


# ----------------------------------------------------------------- entry

def kernel(feat, W1, a1_src, a1_dst, b1, W2, a2_src, a2_dst, b2, fc_W, fc_b,
           edge_index, batch, _want_results=False):
    from concourse.bass_utils import run_bass_kernel_spmd

    Ds, in_maps, fcb = _prep(feat, W1, a1_src, a1_dst, b1, W2, a2_src, a2_dst,
                             b2, fc_W, fc_b, edge_index, batch)
    nc = _build(Ds)
    res = run_bass_kernel_spmd(nc, in_maps, core_ids=list(range(N_CORES)))
    out = np.zeros((NUM_GRAPHS, N_CLASS), dtype=np.float32)
    for c in range(N_CORES):
        out += res.results[c]["out_part"]
    out += fcb[None, :]
    if _want_results:
        return out, (nc, in_maps)
    return out


# revision 15
# speedup vs baseline: 1.0318x; 1.0318x over previous
"""GAT (2-layer graph attention network + mean-pool + FC) on 8 Trainium2 cores.

Strategy
--------
Host: permute nodes by in-degree into tiles of 127 nodes (+1 pad slot at
partition 127, which keeps the tail of every bulk gather on a non-negative
index), shard tiles across 8 cores (dst partitioning), and precompute per-edge
gather index streams.

Device (SPMD, one NEFF on 8 cores):
  - node-feature tables in HBM, bf16 rows of 128 elems (256B):
    [h(64) | alpha_src(4) | alpha_dst(4) | pad]; the alpha projections are
    folded into the weights (W @ a), so one matmul yields h and both alphas.
  - layer-1 projection is replicated (each core projects all nodes from a
    per-core-permuted featT so its own tiles come first -> uniform program).
  - edge phase: per dst-tile [128 nodes x D slots], one bulk dma_gather pulls
    each in-edge's source row; attention is computed densely per (node, slot):
    s = a_src[src] + a_dst[dst] (free-axis broadcast), w = exp(leaky_relu(s)),
    x = sum_d w*h / sum_d w.  Padding slots point at a dummy table row with
    a_src = -60 => w ~ e^-60 ~ 0.
  - layer boundary: per-tile PE transpose + projection of x1, slices
    AllGathered into the layer-2 table (layer-2 gather indices use the
    AllGather layout).
  - pooling: per-graph one-hot matmul accumulated in PSUM; each core emits a
    partial (pooled/cnt) @ fc_W; host sums partials and adds fc_b.
"""

import numpy as np
import ml_dtypes

bf16 = ml_dtypes.bfloat16

# problem constants
N_NODES = 50000
N_EDGES = 800000
N_FEAT = 128
N_HID = 64
HEADS = 4
HEAD_DIM = 16
N_CLASS = 10
NUM_GRAPHS = 64
NEG_SLOPE = 0.2

N_CORES = 8
TPC = 50                      # tiles per core
NT = N_CORES * TPC            # 400 global tiles
RPT = 127                     # real nodes per tile (partition 127 is pad)
NPAD = NT * 128               # 51200 node slots
NROW = NPAD + 1               # tables have one extra dummy row
DUMMY = NPAD                  # dummy table row / sentinel slot
BASE = 32768                  # gather base offset (int16 idx wraps around it)
SPC = TPC * 128               # 6400 slots per core
AG_CHUNKS = [(0, 13), (13, 26), (26, 38), (38, 50)]


# ----------------------------------------------------------------- host prep

def _prep(feat, W1, a1_src, a1_dst, b1, W2, a2_src, a2_dst, b2, fc_W, fc_b,
          edge_index, batch):
    src = np.asarray(edge_index[0], dtype=np.int64)
    dst = np.asarray(edge_index[1], dtype=np.int64)
    batch = np.asarray(batch, dtype=np.int64)
    feat = np.asarray(feat, dtype=np.float32)

    deg = np.bincount(dst, minlength=N_NODES)
    order = np.argsort(-deg, kind="stable")          # node ids, desc degree

    # slot position k = 1024*i + 128*c + p ; real nodes at p in [0, 127)
    # real-rank r (0..N_NODES-1) -> position: i = r // 1016, within = r % 1016,
    # c = within // 127, p = within % 127
    rr = np.arange(N_NODES)
    pos_of_rank = ((rr // 1016) * 1024 + ((rr % 1016) // RPT) * 128
                   + (rr % 1016) % RPT)
    slot_node = np.full(NPAD, -1, dtype=np.int64)    # position -> node id
    slot_node[pos_of_rank] = order
    node_pos = np.empty(N_NODES, dtype=np.int64)     # node id -> position
    node_pos[order] = pos_of_rank

    deg_slot = np.zeros(NPAD, dtype=np.int64)
    deg_slot[pos_of_rank] = deg[order]

    Ds = []
    for i in range(TPC):
        Ds.append(int(max(1, deg_slot[1024 * i:1024 * (i + 1)].max())))

    ks = np.arange(NPAD)
    k_i = ks // 1024
    k_c = (ks % 1024) // 128
    k_p = ks % 128

    # r2: chunked-AllGather table row of position k
    # chunk q covers tile range [i0, i1); within chunk: [c][i-i0][p]
    r2map = np.empty(NROW, dtype=np.int64)
    off = 0
    for (i0, i1) in AG_CHUNKS:
        L = i1 - i0
        m = (k_i >= i0) & (k_i < i1)
        r2map[ks[m]] = (off + k_c[m] * (L * 128)
                        + (k_i[m] - i0) * 128 + k_p[m])
        off += N_CORES * L * 128
    r2map[DUMMY] = DUMMY

    # per-core projection order: own tiles (c, 0..TPC-1) first, then others
    g_of = np.empty((N_CORES, N_CORES, TPC), dtype=np.int64)
    for c in range(N_CORES):
        g_of[c, c, :] = np.arange(TPC)
        g = TPC
        for c2 in range(N_CORES):
            if c2 == c:
                continue
            g_of[c, c2, :] = np.arange(g, g + TPC)
            g += TPC

    # r1 per core: local table row of position k:  r1 = p*NT + g
    r1map = np.empty((N_CORES, NROW), dtype=np.int64)
    for c in range(N_CORES):
        r1map[c, :NPAD] = k_p * NT + g_of[c, k_c, k_i]
        r1map[c, DUMMY] = DUMMY

    # per-dst edge lists -> SRC position matrix [NPAD, Dmax]
    Dmax = max(Ds)
    kd = node_pos[dst]
    ksrc = node_pos[src]
    ordr = np.argsort(kd, kind="stable")
    kd_s = kd[ordr]
    ks_s = ksrc[ordr]
    starts = np.searchsorted(kd_s, np.arange(NPAD))
    dpos = np.arange(N_EDGES) - starts[kd_s]
    SRC = np.full((NPAD, Dmax), DUMMY, dtype=np.int64)
    SRC[kd_s, dpos] = ks_s

    # idx streams (int16, relative to BASE) in gather order g' = d*128 + p:
    # idx tile [16, 8D] wrapped (val(t*16+q) at [q, t]), replicated to 128.
    SD = sum(Ds)
    idx1 = np.empty((N_CORES, 128, 8 * SD), dtype=np.int16)
    idx2 = np.empty((N_CORES, 128, 8 * SD), dtype=np.int16)
    bat = np.empty((N_CORES, 128, TPC), dtype=bf16)
    off = 0
    for i in range(TPC):
        D = Ds[i]
        for c in range(N_CORES):
            base_k = 1024 * i + 128 * c
            blk = SRC[base_k:base_k + 128, :D]           # [128(p), D]
            r2v = (r2map[blk] - BASE).astype(np.int16)
            r1v = (r1map[c][blk] - BASE).astype(np.int16)
            t2 = np.tile(r2v.T.reshape(8 * D, 16).T, (8, 1))
            t1 = np.tile(r1v.T.reshape(8 * D, 16).T, (8, 1))
            idx2[c][:, 8 * off:8 * (off + D)] = t2
            idx1[c][:, 8 * off:8 * (off + D)] = t1
            nodes = slot_node[base_k:base_k + 128]
            bv = np.where(nodes >= 0,
                          batch[np.clip(nodes, 0, N_NODES - 1)],
                          NUM_GRAPHS).astype(np.float32)
            bat[c][:, i] = bv.astype(bf16)
        off += D

    # featT per core: columns ordered by per-core projection tile order
    feat_slot = np.zeros((NPAD, N_FEAT), dtype=np.float32)
    valid = slot_node >= 0
    feat_slot[valid] = feat[slot_node[valid]]
    feat_slot_bf = np.ascontiguousarray(feat_slot.astype(bf16))
    featT = np.empty((N_CORES, N_FEAT, NPAD), dtype=bf16)
    base128 = np.arange(128)
    for c in range(N_CORES):
        kidx = np.empty(NPAD, dtype=np.int64)
        g = 0
        for i in range(TPC):
            kidx[128 * g:128 * (g + 1)] = 1024 * i + 128 * c + base128
            g += 1
        for c2 in range(N_CORES):
            if c2 == c:
                continue
            for i in range(TPC):
                kidx[128 * g:128 * (g + 1)] = 1024 * i + 128 * c2 + base128
                g += 1
        featT[c] = np.ascontiguousarray(feat_slot_bf[kidx].T)

    # folded weights
    def a_mat(a):
        m = np.zeros((N_HID, HEADS), dtype=np.float64)
        a = np.asarray(a, dtype=np.float64)
        for h in range(HEADS):
            m[HEAD_DIM * h:HEAD_DIM * (h + 1), h] = a[h]
        return m

    W1_ = np.asarray(W1, dtype=np.float64)
    W2_ = np.asarray(W2, dtype=np.float64)
    W1e = np.concatenate(
        [W1_, W1_ @ a_mat(a1_src), W1_ @ a_mat(a1_dst)], axis=1).astype(bf16)
    W2e = np.concatenate(
        [W2_, W2_ @ a_mat(a2_src), W2_ @ a_mat(a2_dst)], axis=1).astype(bf16)

    B1T = np.ascontiguousarray(
        np.broadcast_to(np.asarray(b1, np.float32), (128, N_HID)))
    B2T = np.ascontiguousarray(
        np.broadcast_to(np.asarray(b2, np.float32), (128, N_HID)))
    GIOTA = np.ascontiguousarray(np.broadcast_to(
        np.arange(NUM_GRAPHS, dtype=np.float32),
        (128, NUM_GRAPHS)).astype(bf16))
    IDENT = np.eye(128, dtype=np.float32).astype(bf16)
    cnt = np.bincount(batch, minlength=NUM_GRAPHS).astype(np.float32)
    CNTINV = np.ascontiguousarray(np.broadcast_to(
        (1.0 / np.maximum(cnt, 1.0)).astype(np.float32),
        (N_HID, NUM_GRAPHS)))
    FCW = np.asarray(fc_W, dtype=np.float32)
    fcb = np.asarray(fc_b, dtype=np.float32)
    C02 = np.full((128, 1), NEG_SLOPE, dtype=np.float32).astype(bf16)

    in_maps = []
    for c in range(N_CORES):
        in_maps.append({
            "featT": featT[c], "idx1": idx1[c], "idx2": idx2[c],
            "bat": np.ascontiguousarray(bat[c]),
            "W1e": W1e, "W2e": W2e, "B1T": B1T, "B2T": B2T,
            "GIOTA": GIOTA, "IDENT": IDENT, "CNTINV": CNTINV, "FCW": FCW,
            "C02": C02,
        })
    return tuple(Ds), in_maps, fcb


# ------------------------------------------------------------ device program

_BUILD_CACHE = {}


def _build(Ds):
    import concourse.bacc as bacc
    import concourse.mybir as mybir
    import concourse.tile as tile

    if Ds in _BUILD_CACHE:
        return _BUILD_CACHE[Ds]

    SD = sum(Ds)
    Dmax = max(Ds)
    f32 = mybir.dt.float32
    b16 = mybir.dt.bfloat16

    nc = bacc.Bacc("TRN2", target_bir_lowering=False, debug=False,
                   num_devices=N_CORES, num_swdge_queues=4)
    qn_state = [0]
    featT = nc.dram_tensor("featT", [N_FEAT, NPAD], b16, kind="ExternalInput")
    idx1 = nc.dram_tensor("idx1", [128, 8 * SD], mybir.dt.int16,
                          kind="ExternalInput")
    idx2 = nc.dram_tensor("idx2", [128, 8 * SD], mybir.dt.int16,
                          kind="ExternalInput")
    bat = nc.dram_tensor("bat", [128, TPC], b16, kind="ExternalInput")
    W1e_t = nc.dram_tensor("W1e", [N_FEAT, 72], b16, kind="ExternalInput")
    W2e_t = nc.dram_tensor("W2e", [N_HID, 72], b16, kind="ExternalInput")
    B1T_t = nc.dram_tensor("B1T", [128, N_HID], f32, kind="ExternalInput")
    B2T_t = nc.dram_tensor("B2T", [128, N_HID], f32, kind="ExternalInput")
    GIOTA_t = nc.dram_tensor("GIOTA", [128, NUM_GRAPHS], b16,
                             kind="ExternalInput")
    IDENT_t = nc.dram_tensor("IDENT", [128, 128], b16, kind="ExternalInput")
    CNTINV_t = nc.dram_tensor("CNTINV", [N_HID, NUM_GRAPHS], f32,
                              kind="ExternalInput")
    FCW_t = nc.dram_tensor("FCW", [N_HID, N_CLASS], f32, kind="ExternalInput")
    C02_t = nc.dram_tensor("C02", [128, 1], b16, kind="ExternalInput")
    out_part = nc.dram_tensor("out_part", [NUM_GRAPHS, N_CLASS], f32,
                              kind="ExternalOutput")

    table1 = nc.dram_tensor("table1", [NROW, 128], b16, kind="Internal")
    table2 = nc.dram_tensor("table2", [NROW, 128], b16, kind="Internal",
                            addr_space="Shared")
    cc_in = nc.dram_tensor("cc_in", [SPC, 128], b16, kind="Internal")

    with tile.TileContext(nc) as tc:
        with tc.tile_pool(name="consts", bufs=1) as cpool, \
             tc.tile_pool(name="persist", bufs=1) as ppool:
            W1e_sb = cpool.tile([N_FEAT, 72], b16)
            nc.sync.dma_start(W1e_sb[:], W1e_t[:])
            W2e_sb = cpool.tile([N_HID, 72], b16)
            nc.sync.dma_start(W2e_sb[:], W2e_t[:])
            B1T_sb = cpool.tile([128, N_HID], f32)
            nc.sync.dma_start(B1T_sb[:], B1T_t[:])
            B2T_sb = cpool.tile([128, N_HID], f32)
            nc.sync.dma_start(B2T_sb[:], B2T_t[:])
            GIOTA_sb = cpool.tile([128, NUM_GRAPHS], b16)
            nc.sync.dma_start(GIOTA_sb[:], GIOTA_t[:])
            IDENT_sb = cpool.tile([128, 128], b16)
            nc.sync.dma_start(IDENT_sb[:], IDENT_t[:])
            CNTINV_sb = cpool.tile([N_HID, NUM_GRAPHS], f32)
            nc.sync.dma_start(CNTINV_sb[:], CNTINV_t[:])
            FCW_sb = cpool.tile([N_HID, N_CLASS], f32)
            nc.sync.dma_start(FCW_sb[:], FCW_t[:])
            bat_sb = cpool.tile([128, TPC], b16)
            nc.sync.dma_start(bat_sb[:], bat[:])
            C02_sb = cpool.tile([128, 1], b16)
            nc.sync.dma_start(C02_sb[:], C02_t[:])

            # dummy-row patch: h = 0, a_src = -60, a_dst = 0
            dpat = cpool.tile([1, 128], b16)
            nc.vector.memset(dpat[:], 0.0)
            nc.vector.memset(dpat[:, 64:68], -60.0)
            nc.sync.dma_start(table1[DUMMY:DUMMY + 1, :], dpat[:])
            nc.sync.dma_start(table2[DUMMY:DUMMY + 1, :], dpat[:])

            x1T_all = ppool.tile([N_HID, SPC], b16)
            adst1_all = ppool.tile([128, TPC * 4], b16)
            adst2_all = ppool.tile([128, TPC * 4], b16)

            # -------- phase A: layer-1 projection (replicated, NT tiles)
            t1v = table1[0:NPAD, :].rearrange("(p g) e -> p g e", p=128)
            with tc.tile_pool(name="projA", bufs=4) as apool, \
                 tc.tile_pool(name="psumA", bufs=8, space="PSUM") as apsum:
                BT = 16
                for b0 in range(0, NT, BT):
                    blen = min(BT, NT - b0)
                    chunk = apool.tile([N_FEAT, BT * 128], b16, tag="chunk")
                    nc.sync.dma_start(
                        chunk[:, :blen * 128],
                        featT[:, 128 * b0:128 * (b0 + blen)])
                    stag = apool.tile([128, BT * 72], b16, tag="stag")
                    for q0 in range(0, blen, 4):
                        qlen = min(4, blen - q0)
                        ps = apsum.tile([128, 4 * 72], f32, space="PSUM",
                                        tag="psA")
                        for t in range(qlen):
                            g = b0 + q0 + t
                            nc.tensor.matmul(
                                ps[:, 72 * t:72 * (t + 1)],
                                lhsT=chunk[:, 128 * (q0 + t):
                                           128 * (q0 + t + 1)],
                                rhs=W1e_sb[:], start=True, stop=True)
                            if g < TPC:
                                nc.scalar.copy(
                                    adst1_all[:, 4 * g:4 * (g + 1)],
                                    ps[:, 72 * t + 68:72 * t + 72])
                        if (q0 // 4) % 2 == 0:
                            nc.vector.tensor_copy(
                                stag[:, 72 * q0:72 * (q0 + qlen)],
                                ps[:, :72 * qlen])
                        else:
                            nc.scalar.copy(
                                stag[:, 72 * q0:72 * (q0 + qlen)],
                                ps[:, :72 * qlen])
                    nc.sync.dma_start(
                        t1v[:, b0:b0 + blen, 0:72],
                        stag[:, :blen * 72].rearrange(
                            "p (g e) -> p g e", g=blen))

            # per-tile edge aggregation (dense per-node slots)
            def edge_tile(epool, table, idx_t, off8, D, adst_all, i, BT_sb):
                idx_sb = epool.tile([128, 8 * Dmax], mybir.dt.int16,
                                    tag="idx")
                nc.sync.dma_start(idx_sb[:, :8 * D],
                                  idx_t[:, off8:off8 + 8 * D])
                g_t = epool.tile([128, Dmax * 128], b16, tag="gt")
                gv = g_t[:, :D * 128].rearrange("p (d e) -> p d e", d=D)
                # chunked multi-queue gather (single_packet caps at 1024 idx)
                for c0 in range(0, D, 8):
                    cl = min(8, D - c0)
                    nc.gpsimd.dma_gather(
                        out_ap=gv[:, c0:c0 + cl, :],
                        in_ap=table[BASE:, :],
                        idxs_ap=idx_sb[:, 8 * c0:8 * (c0 + cl)],
                        num_idxs=128 * cl, num_idxs_reg=128 * cl,
                        elem_size=128, single_packet=True,
                        queue_num=qn_state[0])
                    qn_state[0] = (qn_state[0] + 1) % 4
                s = epool.tile([128, Dmax * 4], b16, tag="s")
                nc.vector.tensor_tensor(
                    out=s[:, :D * 4].rearrange("p (d h) -> p d h", d=D),
                    in0=gv[:, :, 64:68],
                    in1=adst_all[:, 4 * i:4 * (i + 1)].unsqueeze(1)
                        .broadcast_to([128, D, 4]),
                    op=mybir.AluOpType.add)
                t_ = epool.tile([128, Dmax * 4], b16, tag="t_")
                nc.vector.tensor_tensor(
                    out=t_[:, :D * 4], in0=s[:, :D * 4],
                    in1=C02_sb[:, 0:1].to_broadcast([128, D * 4]),
                    op=mybir.AluOpType.mult)
                s2 = epool.tile([128, Dmax * 4], b16, tag="s2")
                nc.vector.tensor_tensor(out=s2[:, :D * 4], in0=s[:, :D * 4],
                                        in1=t_[:, :D * 4],
                                        op=mybir.AluOpType.max)
                # combined [h*w | w] tile, folded log2 along d (unit stride)
                mw = epool.tile([128, Dmax * 68], b16, tag="mw")
                mv = mw[:, :D * 68].rearrange("p (d e) -> p d e", d=D)
                nc.scalar.activation(mv[:, :, 64:68], s2[:, :D * 4],
                                     mybir.ActivationFunctionType.Exp)
                nc.vector.tensor_tensor(
                    out=mv[:, :, 0:64].rearrange("p d (h c) -> p d h c",
                                                 h=HEADS),
                    in0=gv[:, :, 0:64].rearrange(
                        "p d (h c) -> p d h c", h=HEADS),
                    in1=mv[:, :, 64:68].unsqueeze(3)
                        .broadcast_to([128, D, 4, 16]),
                    op=mybir.AluOpType.mult)
                cur = D
                while cur > 1:
                    k = (cur + 1) // 2
                    r = cur - k
                    nc.vector.tensor_tensor(
                        out=mw[:, :r * 68], in0=mw[:, :r * 68],
                        in1=mw[:, k * 68:cur * 68],
                        op=mybir.AluOpType.add)
                    cur = k
                rcp = epool.tile([128, 4], f32, tag="rcp")
                nc.vector.reciprocal(rcp[:], mw[:, 64:68])
                xm = epool.tile([128, 64], f32, tag="xm")
                nc.vector.tensor_tensor(
                    out=xm[:].rearrange("p (h c) -> p h c", h=HEADS),
                    in0=mw[:, 0:64].rearrange("p (h c) -> p h c", h=HEADS),
                    in1=rcp[:].unsqueeze(2).broadcast_to([128, 4, 16]),
                    op=mybir.AluOpType.mult)
                xb = epool.tile([128, 64], f32, tag="xb")
                nc.vector.tensor_tensor(out=xb[:], in0=xm[:], in1=BT_sb[:],
                                        op=mybir.AluOpType.add)
                x = epool.tile([128, 64], b16, tag="x")
                nc.scalar.activation(x[:], xb[:],
                                     mybir.ActivationFunctionType.Relu)
                return x

            # -------- phases B+C: layer-1 edge + layer-2 projection
            # per tile, with the layer-2-table AllGather chunked in
            with tc.tile_pool(name="edge1", bufs=6) as epool, \
                 tc.tile_pool(name="psumE1", bufs=2, space="PSUM") as epsum:
                off8 = 0
                ag_off = 0
                for i in range(TPC):
                    D = Ds[i]
                    x = edge_tile(epool, table1, idx1, off8, D,
                                  adst1_all, i, B1T_sb)
                    psT = epsum.tile([N_HID, 128], b16, space="PSUM",
                                     tag="psT")
                    nc.tensor.transpose(psT[:], x[:], IDENT_sb[:])
                    nc.scalar.copy(
                        x1T_all[:, 128 * i:128 * (i + 1)], psT[:])
                    psH = epsum.tile([128, 72], f32, space="PSUM", tag="psH")
                    nc.tensor.matmul(
                        psH[:], lhsT=x1T_all[:, 128 * i:128 * (i + 1)],
                        rhs=W2e_sb[:], start=True, stop=True)
                    h2 = epool.tile([128, 128], b16, tag="h2")
                    nc.scalar.copy(h2[:, 0:72], psH[:])
                    nc.scalar.copy(adst2_all[:, 4 * i:4 * (i + 1)],
                                   psH[:, 68:72])
                    nc.sync.dma_start(cc_in[128 * i:128 * (i + 1), :], h2[:])
                    off8 += 8 * D
                    for (i0, i1) in AG_CHUNKS:
                        if i == i1 - 1:
                            L = i1 - i0
                            nc.gpsimd.collective_compute(
                                "AllGather", mybir.AluOpType.bypass,
                                replica_groups=[list(range(N_CORES))],
                                ins=[cc_in[128 * i0:128 * i1, :]],
                                outs=[table2[ag_off:
                                             ag_off + N_CORES * L * 128, :]])
                            ag_off += N_CORES * L * 128

            # -------- phase E: layer-2 edge + pooling
            with tc.tile_pool(name="edge2", bufs=6) as epool, \
                 tc.tile_pool(name="psumP", bufs=1, space="PSUM") as ppsum:
                ps_pool = ppsum.tile([N_HID, NUM_GRAPHS], f32, space="PSUM")
                off8 = 0
                for i in range(TPC):
                    D = Ds[i]
                    x = edge_tile(epool, table2, idx2, off8, D,
                                  adst2_all, i, B2T_sb)
                    P_pool = epool.tile([128, NUM_GRAPHS], b16, tag="pp")
                    nc.vector.tensor_tensor(
                        out=P_pool[:],
                        in0=bat_sb[:, i:i + 1].to_broadcast(
                            [128, NUM_GRAPHS]),
                        in1=GIOTA_sb[:], op=mybir.AluOpType.is_equal)
                    nc.tensor.matmul(ps_pool[:], lhsT=x[:], rhs=P_pool[:],
                                     start=(i == 0), stop=(i == TPC - 1))
                    off8 += 8 * D

                # -------- phase F: scale + FC
                pooledS = epool.tile([N_HID, NUM_GRAPHS], f32)
                nc.vector.tensor_tensor(out=pooledS[:], in0=ps_pool[:],
                                        in1=CNTINV_sb[:],
                                        op=mybir.AluOpType.mult)
                ps_o = ppsum.tile([NUM_GRAPHS, N_CLASS], f32, space="PSUM")
                nc.tensor.matmul(ps_o[:], lhsT=pooledS[:], rhs=FCW_sb[:],
                                 start=True, stop=True)
                outsb = epool.tile([NUM_GRAPHS, N_CLASS], f32)
                nc.vector.tensor_copy(outsb[:], ps_o[:])
                nc.sync.dma_start(out_part[:], outsb[:])

    nc.compile()
    _BUILD_CACHE[Ds] = nc
    return nc


# ----------------------------------------------------------------- entry

def kernel(feat, W1, a1_src, a1_dst, b1, W2, a2_src, a2_dst, b2, fc_W, fc_b,
           edge_index, batch, _want_results=False):
    from concourse.bass_utils import run_bass_kernel_spmd

    Ds, in_maps, fcb = _prep(feat, W1, a1_src, a1_dst, b1, W2, a2_src, a2_dst,
                             b2, fc_W, fc_b, edge_index, batch)
    nc = _build(Ds)
    res = run_bass_kernel_spmd(nc, in_maps, core_ids=list(range(N_CORES)))
    out = np.zeros((NUM_GRAPHS, N_CLASS), dtype=np.float32)
    for c in range(N_CORES):
        out += res.results[c]["out_part"]
    out += fcb[None, :]
    if _want_results:
        return out, (nc, in_maps)
    return out
